# revision 49
# baseline (speedup 1.0000x reference)
"""MinGRU kernel for Trainium2 (8 NeuronCores, Bass/Tile) — final (v17).

Measured 80.4 us (v18, era-matched -0.5 vs v17's 80.9/81.2; v17 best
79.4 in an earlier cool window) vs 82.5-82.9 (v3 baseline), rel err
1.38e-3. Absolute numbers drift +-2us with device thermal state.

Reference computation (B=4, L=8192, D=512, fp32):
    gates = sigmoid(x @ Wg.T + bg)
    cands = tanh(x @ Wc.T + bc)
    h_t   = (1 - g_t) * h_{t-1} + g_t * c_t   (scan along L, h_0 = 0)

Sharding: core c -> (batch b = c//2, channel half = c%2). Each core computes
its batch's full L range for 256 of the 512 output channels; the scan along L
is per (b, channel) so no cross-core communication is needed.

Measured engine budget per core (v1 trace): PE 54.6us (fixed roofline:
131072 PE rows), DVE scan 2.09ns/elem + bneg STT 1.04ns/elem over 16384
elems/lane = 51us payload — DVE is the co-bottleneck, so the design
minimizes DVE instruction count and keeps the dependency graph
single-chain (v2's DVE<->GpSimd ping-pong doubled semaphore costs and
regressed; GpSimd tensor ops run at ~2ns/elem + ~570ns/op and are not
worth it).

v3 vs v1 (90.5us):
  * x and W cast to fp16 on the host: input DMA halves (16.8 -> 8.9 MB/core);
    the x feed (33us queue-wall) ducks well under the PE roofline.
  * Scan units of 2048 tokens: one STT + one scan per (unit, e-tile) with
    matmul/ACT filling the unit in 1024-token halves ([128,1024] fp32 PSUM
    tiles = 2 banks, tags g/c x bufs 2 = 8 banks, still double-buffered).
    Fewer DVE ops -> less fixed overhead and fewer semaphores.
  * Activations read the full 1024-token PSUM tile in one instruction.
  * 26 warm-up matmuls on a zeroed dummy tile while the first weight/x DMAs
    fly: PE_HAM releases the 4/8 cold clock gate before real matmuls start,
    and the PE is never idle long enough to re-throttle.
  * wg ships in two pieces (dc chunk 0 first) so the first real matmul only
    waits for 64KB of weights plus the first x segment.
  * Segment ramp [512, 1024, 1536, 2048, ...] matched to the x queue's
    ~0.37 MB/us delivery rate so the PE rarely outruns the feed.
  * -bg negated on the host; h stored fp16 [2, 128, L] and upcast on host.

v10 vs v3 (82.5us):
  * Weights ride the Scalar engine's HWDGE queue, landing in parallel
    with x segment 0 on the sync ring: the DMA rate ramps slowly over the
    first ~5us, so serializing the ~1MB front (weights + x0) on one ring
    cost ~2us of first-activation latency.
  * Warm-up burst 16 matmuls, retimed to end at the first x arrival
    (~12.4us: the DMA rate ramps slowly for ~4us regardless of line
    size, so x seg 0 lands later than v3 assumed; 13 left a ~1us PE
    idle gap before the real stream).
  * x packed per segment in DRAM ([128, NDC*lt] contiguous blocks):
    seg DMAs move 4-16KB contiguous lines per partition instead of
    2*lt-byte lines. Measured neutral on this box (the feed is
    HBM-share/ramp-bound, not line-size-bound) but strictly fewer
    descriptors.
  * Per-unit h stores dispatch at the TOP of the next unit, and the last
    unit stores each e-tile right after its scan (et0 store overlaps the
    et1 scan instead of waiting for it).
  * x segment 0 ships as two dc-pair tiles (contiguous DRAM ranges in
    the per-seg-packed layout): the dc0/1 matmuls start at half-arrival
    during the slow DMA ramp, shifting the whole PE/act curve ~1us left
    (the mid-ramp A(T)+remaining-DVE bound scales with PE start).
  * (v17) bias DMA moved AFTER the x0 halves on the sync ring: its
    dispatch+2KB transfer ahead of x0a cost ~0.5us at the most
    latency-critical point; bias is not needed until the first act.
  * (v18) The first 2048 unit is split into two 1024 units
    ([512,1024,1024,1024,2048,2048,512], still 9 psegs = 36 acts, the
    known Scalar sweet spot): NSPLIT-trace analysis showed dc-splits
    move the front -0.5us but leave last-scan-end byte-identical,
    because the binding 2.7us stall is the scan waiting the unit's LAST
    pseg act (bound by the seg's full DMA arrival) -- only a smaller
    scan unit moves it. Measured 80.38 vs 80.9/81.2 for the old
    schedule in adjacent same-thermal runs; last-scan-end 75.75us vs
    76.44 (first movement of that milestone all session). Splitting the
    NEXT 2048 as well ([512,1024x5,2048,512]) chased the stall to the
    final 2048 (new 2.1us gap at 57.5us) and measured 81.4 -- late
    units absorb backlog, so one split at the first binding unit is
    the optimum.
  * NSPLIT below generalizes the x dc-pair split to the first N
    segments. NSPLIT=3 (+ moving the bn tiles from the 4-buf work pool
    to the 1-buf mpool, safe since tt->scan is same-engine serial, to
    free the needed SBUF) compiled, fit, and measured 80.9 / rel err
    1.38e-3 -- inside the NSPLIT=1 band (79.4/80.9/81.2), no
    demonstrated win under thermal noise. Shipping NSPLIT=1, the config
    with three validated samples and the only sub-80 measurements.

Dead ends measured this session (do not retry):
  * fp8e4m3 DoubleRow matmuls: same 379ns/instruction as fp16 on HW
    (cost model's 0.5 cycles/row is wrong here); 6 DR matmuls per bank
    vs 4 fp16 -> PE 1.5x slower, 110us total. Precision (3-term hi+lo,
    shared scale) was fine (5.4e-3) -- it's purely a speed dead end.
  * Merging both e-tiles into one scan via boundary columns (a=1/bn=0
    pass-through, a=0/bn=-carry injection): works numerically but
    serializes both e-tiles' activations into the scan critical path,
    losing the scan(et0) || act(et1) overlap: 91us.
  * scalar_tensor_tensor for bn on DVE: 1.18 ns/elem vs 1.0 for
    ts_sub(4x)+tt_mul(2x), net +1.8us even with fewer instructions.
  * GpSimd cannot run TensorScalarPtr at all (ISA engine check).
  * Segment schedule changes move <=1.5us either way; PE (6.67 ns/tok)
    and DVE (~6.8 ns/tok effective) are rate-matched streams, so
    end ~= first-scan-start + DVE stream time regardless of schedule.
  * Splitting the last pseg's activations into 512-token halves (to cut
    the act-tail the scan waits on): +15us. Act/semaphore count on the
    Scalar queue is hyper-sensitive; 36 acts is the sweet spot (v5a's
    48 acts already cost +9us of Scalar event time).
  * am1 = a-1 as a Scalar Identity-act (bias -1) to offload the DVE ts:
    +3us — same 48-act sensitivity.
  * h stores on the scalar DMA ring (1-unit delay): +2.3us — the store's
    scan-done wait head-of-line blocks later acts on the Scalar queue.

Known framework overhead (not reachable from kernel code): the bacc
epilogue zeros every allocated event semaphore with INDIVIDUAL
EVENT_SEMAPHORE $S[n]=0 instructions (~250 of them, ~6us of chained
clears, roughly one semaphore per logical tile edge), though a
RANGE_CLEAR instruction exists. Only ~1us of it lands inside the
measured exec window; shrinking it would need a bacc change or far
fewer logical tiles. The ~6.8us preamble (all-engine barrier rounds +
per-engine table loads) is similarly fixed.
"""

import os
import sys

sys.path.insert(0, "/opt/trn_rl_repo")

import numpy as np

import concourse.bacc as bacc
import concourse.bass as bass
import concourse.mybir as mybir
from concourse.bass_utils import run_bass_kernel_spmd
from concourse.tile import TileContext

B, L, D = 4, 8192, 512
NCORES = 8
EH = D // 2          # output channels per core
NET = EH // 128      # e-tiles per core (2)
NDC = D // 128       # contraction chunks (4)
NSUB = 512           # one fp32 PSUM bank of tokens (matmul N limit)
PSEG = 1024          # tokens per PSUM tile / ACT instruction
# Scan units: one STT + scan per unit; matmul/ACT work in <=1024 chunks.
# The x feed delivers ~0.37 MB/us on one HWDGE ring (~HBM roofline share)
# vs the PE's 0.30 MB/us consumption, so the ramp below is feed-matched:
# starting the PE earlier or splitting x across DMA rings was measured to
# starve the early segments (the SDMA engines and HBM are shared).
SEGS = [512, 1024, 1024, 1024, 2048, 2048, 512]
assert sum(SEGS) == L
MAXSEG = max(SEGS)

FP32 = mybir.dt.float32
F16 = mybir.dt.float16
_last_results = None

# Sized so the burst (a) lasts >3.4us so PE_HAM flips to full clock DURING
# warm-up, and (b) ends just before x segment 0 lands (~11.5-12.4us) so it
# never blocks the real stream; the residual sub-3.4us idle gap is too
# short to re-throttle.
N_WARMUP_MM = 16
# Measured (twice): ANY GpSimd Q7 tensor op running concurrently with DVE
# work inflates DVE op durations ~20-30% (SBUF port contention from the
# software engine), a strict net loss since DVE is the pacer. All
# elementwise work therefore stays on DVE; GpSimd only does DMA.


def build_nc() -> bass.Bass:
    # Bacc (not plain Bass): its compile() runs move_matmul_waits_to_ldweights
    # and generate_event_semaphores, which split multi-sem waits to satisfy the
    # TRN2 per-instruction wait-slot limits walrus enforces.
    nc = bacc.Bacc()

    # x is packed per segment: each seg is one contiguous [NDC*lt] block per
    # partition, so every seg DMA moves 4-16KB contiguous lines instead of
    # 2*lt-byte lines (the [128, NDC, L] layout split lines at the dc dim,
    # capping the early feed at ~0.25 MB/us for the small ramp segments).
    xr = nc.dram_tensor("xr", [128, NDC * L], F16, kind="ExternalInput")
    wg = nc.dram_tensor("wg", [128, NDC, EH], F16, kind="ExternalInput")
    wc = nc.dram_tensor("wc", [128, NDC, EH], F16, kind="ExternalInput")
    # bias packed [128, 4]: cols 0..1 = -bg per e-tile, 2..3 = bc per e-tile
    bias = nc.dram_tensor("bias", [128, 2 * NET], FP32, kind="ExternalInput")
    h = nc.dram_tensor("h", [NET, 128, L], F16, kind="ExternalOutput")
    h_pel = h.rearrange("e p l -> p e l")

    op = mybir.AluOpType
    act = mybir.ActivationFunctionType

    with TileContext(nc) as tc:
        with (
            tc.tile_pool(name="consts", bufs=1) as consts,
            tc.tile_pool(name="xpool", bufs=3) as xpool,
            tc.tile_pool(name="x0pool", bufs=1) as x0pool,
            tc.tile_pool(name="work", bufs=4) as work,
            tc.tile_pool(name="mpool", bufs=1) as mpool,
            tc.tile_pool(name="hpool", bufs=3) as hpool,
            tc.tile_pool(name="psum", bufs=2, space="PSUM") as psum,
        ):
            # PE warm-up: zero a dummy tile, then issue back-to-back matmuls
            # on it while the first weight/x DMAs are still in flight, so
            # PE_HAM releases the 4/8 cold clock gate before the real stream.
            dummy = consts.tile([128, 128], F16)
            nc.vector.memset(dummy, 0.0)
            warm_ps = psum.tile([128, PSEG], FP32, tag="pg", name="warm")
            for _ in range(N_WARMUP_MM):
                nc.tensor.matmul(
                    warm_ps[:, 0:128], dummy, dummy, start=True, stop=True
                )

            # Everything rides the sync HWDGE ring (no SWDGE at all: an idle
            # GpSimd skips its expensive dge_drain in the epilogue). Sync
            # queue order: x0a -> x0b -> bias -> x1.. -> h stores. The bias
            # (2KB) rides AFTER the x0 halves: its dispatch+transfer ahead of
            # x0a cost ~0.5us at the most latency-critical point, and it is
            # not needed until the first activation (~2us later).
            bias_sb = consts.tile([128, 2 * NET], FP32)
            wg_sb = consts.tile([128, NDC, EH], F16)
            wc_sb = consts.tile([128, NDC, EH], F16)
            # Weights ride the Scalar engine's HWDGE queue so they land in
            # parallel with x segment 0 on the sync ring (the DMA rate ramps
            # slowly in the first ~5us; serializing ~1MB there costs ~2us).
            nc.scalar.dma_start(wg_sb, wg[:])
            nc.scalar.dma_start(wc_sb, wc[:])
            # Segments 0-2 ship as two dc-pair tiles each: their dc0/1
            # matmuls start when the first half lands (the PE otherwise
            # stalls on each early segment's tail during the DMA ramp; seg 2
            # is the binding unit of the end-time model). Later segments
            # arrive ahead of the PE, so splitting them buys nothing.
            NSPLIT = 1
            xs_tiles = {}
            x_tiles = [None] * len(SEGS)
            for t, lt in enumerate(SEGS[:NSPLIT]):
                lh = lt * NDC // 2
                xs_tiles[t] = [
                    x0pool.tile([128, lh], F16, tag=f"xs{t}{i}", name=f"xs{t}{i}")
                    for i in range(2)
                ]
            for t, lt in enumerate(SEGS[NSPLIT:], start=NSPLIT):
                x_tiles[t] = xpool.tile(
                    [128, NDC * MAXSEG], F16, tag="x", name=f"x_{t}"
                )[:, : NDC * lt]
            xo = 0
            for t, lt in enumerate(SEGS):
                if t in xs_tiles:
                    lh = lt * NDC // 2
                    nc.sync.dma_start(xs_tiles[t][0], xr[:, xo : xo + lh])
                    nc.sync.dma_start(xs_tiles[t][1], xr[:, xo + lh : xo + 2 * lh])
                    xo += 2 * lh
                else:
                    nc.sync.dma_start(x_tiles[t], xr[:, xo : xo + NDC * lt])
                    xo += NDC * lt
                if t == 0:
                    nc.sync.dma_start(bias_sb, bias[:])

            carry = [None] * NET  # [128, 1] AP of the previous h column
            pending_store = None  # (l0, lt, h2) delayed one unit so the
            # gpsimd queue never head-of-line blocks its bn ops on a scan

            l0 = 0
            last_t = len(SEGS) - 1
            for t, lt in enumerate(SEGS):
                x_sb = x_tiles[t]  # None for split segments (xap handles them)
                h2 = hpool.tile([128, NET, MAXSEG], F16, tag="h", name=f"h_{t}")
                # One store per unit covering both e-tiles, emitted one unit
                # late (sync ring; all x dispatches precede these in program
                # order so stores cannot delay the feed). Dispatched before
                # this unit's compute so the final inline stores can't be
                # head-of-line blocked behind it.
                if pending_store is not None:
                    pl0, plt, ph2 = pending_store
                    nc.sync.dma_start(
                        h_pel[:, :, pl0 : pl0 + plt], ph2[:, :, :plt]
                    )
                pending_store = None if t == last_t else (l0, lt, h2)
                for et in range(NET):
                    esl = slice(et * 128, (et + 1) * 128)
                    a_t = work.tile(
                        [128, MAXSEG], F16, tag=f"a{et}", name=f"a{et}_{t}"
                    )[:, :lt]
                    c_t = work.tile(
                        [128, MAXSEG], F16, tag=f"c{et}", name=f"c{et}_{t}"
                    )[:, :lt]
                    # 1024-token PSUM passes fill the scan unit. Separate
                    # pg/pc tags: a merged 4-bank tile was measured to
                    # serialize the MM stream (+14us on the PE).
                    for p0 in range(0, lt, PSEG):
                        pw = min(PSEG, lt - p0)
                        pg = psum.tile(
                            [128, PSEG], FP32, tag="pg", name=f"pg{et}_{t}_{p0}"
                        )
                        pc = psum.tile(
                            [128, PSEG], FP32, tag="pc", name=f"pc{et}_{t}_{p0}"
                        )
                        for n0 in range(0, pw, NSUB):
                            w = min(NSUB, pw - n0)
                            def xap(dc):
                                if t in xs_tiles:
                                    xf = (dc % 2) * lt + p0 + n0
                                    return xs_tiles[t][dc // 2][:, xf : xf + w]
                                xf = dc * lt + p0 + n0
                                return x_sb[:, xf : xf + w]

                            for dc in range(NDC):
                                nc.tensor.matmul(
                                    pg[:, n0 : n0 + w],
                                    wg_sb[:, dc, esl],
                                    xap(dc),
                                    start=(dc == 0),
                                    stop=(dc == NDC - 1),
                                )
                            for dc in range(NDC):
                                nc.tensor.matmul(
                                    pc[:, n0 : n0 + w],
                                    wc_sb[:, dc, esl],
                                    xap(dc),
                                    start=(dc == 0),
                                    stop=(dc == NDC - 1),
                                )
                        # a = sigmoid(-(z_g + bg)) = 1 - g ; c = tanh(z_c + bc)
                        # (Splitting these per 512 to shave the act-tail was
                        # measured at +15us: act/semaphore count on Scalar is
                        # hyper-sensitive; 36 acts is the sweet spot.)
                        nc.scalar.activation(
                            a_t[:, p0 : p0 + pw], pg[:, :pw], act.Sigmoid,
                            bias=bias_sb[:, et : et + 1], scale=-1.0,
                        )
                        nc.scalar.activation(
                            c_t[:, p0 : p0 + pw], pc[:, :pw], act.Tanh,
                            bias=bias_sb[:, NET + et : NET + et + 1], scale=1.0,
                        )
                    # bneg = (a - 1) * c = -g * c. Two DVE ops instead of the
                    # scalar_tensor_tensor: tensor_scalar runs in 4x mode and
                    # tensor_tensor in 2x mode for fp16 (the STT has no fast
                    # uop and is stuck at 1x) — ~25% cheaper despite being
                    # two instructions, and both are same-engine so no extra
                    # cross-engine semaphores.
                    am1 = mpool.tile(
                        [128, MAXSEG], F16, tag=f"m{et}", name=f"m{et}_{t}"
                    )[:, :lt]
                    nc.vector.tensor_scalar_sub(am1, a_t, 1.0)
                    bn_t = work.tile(
                        [128, MAXSEG], F16, tag=f"b{et}", name=f"b{et}_{t}"
                    )[:, :lt]
                    nc.vector.tensor_mul(bn_t, am1, c_t)
                    # h = a * h_prev - bneg  (fp32 state in HW, fp16 storage)
                    init = 0.0 if carry[et] is None else carry[et]
                    nc.vector.tensor_tensor_scan(
                        h2[:, et, :lt], a_t, bn_t, init, op.mult, op.subtract
                    )
                    carry[et] = h2[:, et, lt - 1 : lt]
                    # Last unit: store each e-tile as soon as its scan is
                    # done so the et0 store overlaps the et1 scan.
                    if t == last_t:
                        nc.sync.dma_start(
                            h_pel[:, et, l0 : l0 + lt], h2[:, et, :lt]
                        )
                l0 += lt
    return nc


def _in_maps(x, Wg, bg, Wc, bc):
    maps = []
    xr = {}
    for c in range(NCORES):
        b, eh = c // 2, c % 2
        e0 = eh * EH
        if b not in xr:
            # [L, D] -> [D, L] -> [dc, p, L] -> [p, dc, L] fp16, then packed
            # per segment: [p, NDC*lt] contiguous blocks concatenated.
            xb = x[b].T.reshape(NDC, 128, L).transpose(1, 0, 2).astype(np.float16)
            blocks, lq = [], 0
            for lt in SEGS:
                blocks.append(xb[:, :, lq : lq + lt].reshape(128, NDC * lt))
                lq += lt
            xr[b] = np.ascontiguousarray(np.concatenate(blocks, axis=1))
        bias_pack = np.concatenate(
            [
                (-bg[e0 : e0 + EH]).reshape(NET, 128).T,
                bc[e0 : e0 + EH].reshape(NET, 128).T,
            ],
            axis=1,
        ).astype(np.float32)
        maps.append(
            {
                "xr": xr[b],
                "wg": Wg[e0 : e0 + EH].T.reshape(NDC, 128, EH)
                .transpose(1, 0, 2).astype(np.float16),
                "wc": Wc[e0 : e0 + EH].T.reshape(NDC, 128, EH)
                .transpose(1, 0, 2).astype(np.float16),
                "bias": np.ascontiguousarray(bias_pack),
            }
        )
    return maps


def kernel(x, Wg, bg, Wc, bc):
    global _last_results
    x = np.asarray(x, dtype=np.float32)
    Wg = np.asarray(Wg, dtype=np.float32)
    bg = np.asarray(bg, dtype=np.float32)
    Wc = np.asarray(Wc, dtype=np.float32)
    bc = np.asarray(bc, dtype=np.float32)

    nc = build_nc()
    if not nc.is_finalized():
        nc.finalize()
    res = run_bass_kernel_spmd(
        nc,
        _in_maps(x, Wg, bg, Wc, bc),
        list(range(NCORES)),
        tmpdir=os.environ.get("KERNEL_TMPDIR"),
    )
    _last_results = res

    out = np.empty((B, L, D), dtype=np.float32)
    for b in range(B):
        hb = np.concatenate(
            [
                res.results[2 * b]["h"].reshape(EH, L),
                res.results[2 * b + 1]["h"].reshape(EH, L),
            ],
            axis=0,
        ).astype(np.float32)
        out[b] = hb.T
    return out

